# revision 57
# baseline (speedup 1.0000x reference)
"""DiceBCELossWithTopology fused loss kernel for Trainium2 (8 NeuronCores).

Reference computation (on inputs x, t of shape (64,1,512,512) f32, flattened):
  dice  = 1 - (2*sum(x*t)+1) / (sum(x)+sum(t)+1)
  bce   = mean(-(t*max(log x,-100) + (1-t)*max(log1p(-x),-100)))
  topo  = |n_runs_of_nonzero(x) - 1| / (512*512)
  loss  = 0.5*bce + dice + topo

Strategy (data-parallel over 8 cores, memory-bound):
  Inputs are cast to bf16 on the host (marshalling), halving HBM traffic
  to ~8.4 MB/core.  Each core gets a contiguous 2M-element shard viewed
  as [128, 16384], streamed in ramp-up/taper chunks with the x stream
  prefetched two chunks ahead of t (HWDGE drains FIFO; ACT needs only x).
  Per chunk:
    ACT : L1 = Ln(x + 2^-24), L2 = Ln(-x + (1+EPS2)) into sections of a
          wide rhs tile R.  The biases make -inf impossible -> no clamps.
          ACT (2 passes at ~1 elem/cycle/lane) is the steady bottleneck
          and runs saturated start to finish.
    DVE : pack x and t into matmul layouts (plain 4x-mode copies; no
          slow accumulator ops anywhere).
    PE  : warm-up matmuls at t~0 un-throttle HAM (PE cold clock is 1.2
          GHz vs 2.4 warm) while the first DMA flies; then ONE matmul
          per 126-col sub-chunk: weights = [t-block(126) | ones | zero],
          rhs = [L1 | L2 | x | 1] (379 cols), accumulated into 2
          ping-pong PSUM banks.  Diagonals give sum(t*L1), sum(t*L2),
          sum(x*t); psum ROW 126 (ones weight row) gives sum(L2) and
          sum(x) column sums; the rhs ones column gives sum(t).
  Tail: both psum banks are copied to SBUF concurrently (ACT + DVE),
  rounded to bf16 (psum entries are O(10k) sums, adds <1e-4 error), and
  DMA'd out on separate HWDGE rings; the host adds them and extracts the
  diagonals/row/column.  Keep-alive matmuls between chunk bursts hold
  the PE at its warm 2.4 GHz clock (HAM re-throttles after >3.4us idle).
  Host: float64 reduction over the two per-core psum matrices + exact
  topology (run-start count) from the original f32 data + the last 4
  columns of each core's shard (130*126 = 16380 < 16384) + loss assembly.

Numerics (tolerance 2e-2; these land ~1e-4):
  - Ln(x + 2^-24): only true x==0 affected (-16.6 instead of torch's
    clamped -100; ~1 element in 16.7M -> ~5e-6).
  - Ln(-x + (1+EPS2)): bf16 rounds x in [1-2^-9, 1) UP to 1.0 (~0.2% of
    elements) which would give Ln(0); the bias gives ln(EPS2) instead,
    near the bucket's conditional mean ln(2^-9)-1, and the residual
    bucket bias cancels against the +ln(1+EPS2/(1-x)) smearing of other
    elements.  EPS2 = e^-8.2 balances them (host-emulated ~4e-5).
  - Dice sums: bf16 rounding unbiased; ~1e-6.
  - Topology: computed on host from the ORIGINAL f32 input -> exact.
"""

import numpy as np

# Problem constants (hardcoded per harness contract - no file reads here).
N_CORES = 8
P = 128                      # SBUF partitions
COLS = 16384                 # columns per core: 2M elements / 128
SUBW = 126                   # data columns per matmul sub-chunk (even!)
NSUBS = 130                  # sub-chunks per core
COLS_DEV = SUBW * NSUBS      # 16380 device columns; last 4 done on host
CHUNKS_S = [4, 8, 16, 32, 32, 24, 14]   # per-chunk sub-chunk counts
NCHUNK = len(CHUNKS_S)
TOTAL = 64 * 512 * 512       # 16_777_216 elements
IMAGE_PIXELS = 512 * 512
SMOOTH = 1.0
BCE_WEIGHT = 0.5
TOPOLOGY_WEIGHT = 1.0

B1 = 2.0 ** -24
EPS2 = float(np.exp(-8.2))

N_WARM = 30                  # PE warm-up matmuls (HAM un-throttle)
N_KEEP = 14                  # PE keep-alive matmuls between chunk bursts
WARM_FD = 256

# rhs group layout: [L1 0:126 | L2 126:252 | xb 252:378 | ones 378]
GW = 380                     # group stride (even -> 4B-aligned sections)
NRHS = 379                   # matmul free size

_CACHE = {}


def _build_nc():
    from concourse.bacc import Bacc
    import concourse.mybir as mybir
    from concourse.tile import TileContext

    F32 = mybir.dt.float32
    BF16 = mybir.dt.bfloat16
    AF = mybir.ActivationFunctionType
    OP = mybir.AluOpType
    AX = mybir.AxisListType

    nc = Bacc()
    x_d = nc.dram_tensor("x", [P, COLS], BF16, kind="ExternalInput")
    t_d = nc.dram_tensor("t", [P, COLS], BF16, kind="ExternalInput")
    # bf16 outputs: psum entries are O(10k) sums, bf16 rounding adds <1e-4
    # loss error while halving the tail DMA (its receipt is on the
    # critical path between the last matmul and NEFF end).
    stats_d = nc.dram_tensor("stats", [P, NRHS], BF16, kind="ExternalOutput")
    stats2_d = nc.dram_tensor("stats2", [P, NRHS], BF16, kind="ExternalOutput")

    with TileContext(nc) as tc:
        with tc.tile_pool(name="const", bufs=1) as cpool, \
             tc.tile_pool(name="work", bufs=4) as pool, \
             tc.tile_pool(name="psum", bufs=1, space="PSUM") as psum_pool:

            warmW = cpool.tile([P, P], BF16)
            warmR = cpool.tile([P, WARM_FD], BF16)
            b1c = cpool.tile([P, 1], F32)
            b2c = cpool.tile([P, 1], F32)

            psumB = [psum_pool.tile([P, NRHS], F32, name=f"psumB{i}")
                     for i in range(2)]
            psumW = [psum_pool.tile([P, WARM_FD], F32, name=f"psumW{i}")
                     for i in range(2)]

            # ---- ACT table preload (Ln) - issued before any DMA waits
            nc.vector.memset(warmW[:], 0.0)
            nc.scalar.activation(warmW[:, 0:2], warmW[:, 0:2], AF.Ln)

            # ---- PE warm-up: dense matmul activity from t~0 so HAM
            # un-throttles (1.2 -> 2.4 GHz) before the real matmuls.
            nc.vector.memset(warmR[:], 0.0)
            nc.vector.memset(b1c[:], B1)
            nc.vector.memset(b2c[:], 1.0 + EPS2)
            for w in range(N_WARM):
                nc.tensor.matmul(psumW[w % 2][:], warmW[:], warmR[:],
                                 start=True, stop=True, skip_group_check=True)

            FCMAX = max(CHUNKS_S) * SUBW
            offs = [sum(CHUNKS_S[:k]) * SUBW for k in range(NCHUNK)]
            fcs = [S * SUBW for S in CHUNKS_S]

            # ---- x prefetched two chunks ahead of t: HWDGE drains its ring
            # in FIFO issue order, and ACT (the bottleneck) only needs x.
            # Transfer order: x0, x1, x2, t0, x3, t1, ...
            x_tiles = [None] * NCHUNK

            def fetch_x(k):
                x_tiles[k] = pool.tile([P, FCMAX], BF16, tag="x_t",
                                       name=f"x_t{k}")[:, :fcs[k]]
                nc.sync.dma_start(x_tiles[k], x_d[:, offs[k]:offs[k] + fcs[k]])

            fetch_x(0)
            fetch_x(1)

            s_glob = 0
            for j, S in enumerate(CHUNKS_S):
                FC = fcs[j]
                off = offs[j]
                if j + 2 < NCHUNK:
                    fetch_x(j + 2)
                x_t = x_tiles[j]
                t_t = pool.tile([P, FCMAX], BF16, tag="t_t", name=f"t_t{j}")[:, :FC]
                t4 = pool.tile([P, (FCMAX // SUBW) * P], BF16,
                               tag="t4", name=f"t4_{j}")[:, :S * P]
                R = pool.tile([P, (FCMAX // SUBW) * GW], BF16,
                              tag="R", name=f"R{j}")[:, :S * GW]
                nc.sync.dma_start(t_t, t_d[:, off:off + FC])

                x3 = x_t.rearrange("p (g w) -> p g w", w=SUBW)
                t3 = t_t.rearrange("p (g w) -> p g w", w=SUBW)
                t4r = t4.rearrange("p (g w) -> p g w", w=P)
                R3 = R.rearrange("p (g w) -> p g w", w=GW)

                # ---- ACT: logs (bf16 out); biases avoid -inf, no clamps
                nc.scalar.activation(R3[:, :, 0:SUBW], x3, AF.Ln,
                                     bias=b1c[:, 0:1])
                nc.scalar.activation(R3[:, :, SUBW:2 * SUBW], x3, AF.Ln,
                                     scale=-1.0, bias=b2c[:, 0:1])

                # ---- DVE: 4x-mode packs + tiny memsets
                nc.vector.tensor_copy(R3[:, :, 2 * SUBW:3 * SUBW], x3)
                nc.vector.memset(R3[:, :, 3 * SUBW:3 * SUBW + 1], 1.0)
                nc.vector.tensor_copy(t4r[:, :, 0:SUBW], t3)
                nc.vector.memset(t4r[:, :, SUBW:SUBW + 1], 1.0)
                nc.vector.memset(t4r[:, :, SUBW + 1:SUBW + 2], 0.0)

                # ---- PE: one wide fused matmul per sub-chunk
                for c in range(S):
                    first = s_glob < 2
                    last = s_glob >= NSUBS - 2
                    nc.tensor.matmul(
                        psumB[s_glob % 2][:], t4r[:, c, :],
                        R[:, c * GW:c * GW + NRHS],
                        start=first, stop=last, skip_group_check=True)
                    s_glob += 1
                off += FC

                # ---- PE keep-alives: a chunk's matmuls finish well before
                # the next chunk's ACT does, and a >3.4us PE idle gap makes
                # HAM re-throttle the PE clock to 1.2 GHz.  Filler matmuls
                # in the natural idle window keep it at 2.4 GHz.
                if j < NCHUNK - 3:
                    for w in range(N_KEEP):
                        nc.tensor.matmul(psumW[w % 2][:], warmW[:], warmR[:],
                                         start=True, stop=True,
                                         skip_group_check=True)

            # ---- extraction: ship both accumulated psum banks to the host
            # (bank add + diag/row/col sums are ~3000 flops there).  The two
            # PSUM->SBUF copies run concurrently on ACT and DVE.
            psB_sb = cpool.tile([P, NRHS], BF16)
            psB_sb2 = cpool.tile([P, NRHS], BF16)
            nc.scalar.copy(psB_sb[:], psumB[0][:])
            nc.vector.tensor_copy(psB_sb2[:], psumB[1][:])
            nc.sync.dma_start(stats_d[:], psB_sb[:])
            # second DMA on the ACT HWDGE ring so the two completions overlap
            nc.scalar.dma_start(stats2_d[:], psB_sb2[:])

    nc.finalize()
    return nc


def _get_nc():
    if "nc" not in _CACHE:
        _CACHE["nc"] = _build_nc()
    return _CACHE["nc"]


def _topology_starts(xf: np.ndarray) -> float:
    """Exact count of runs of nonzero elements in xf (1-D, f32)."""
    zeros = np.flatnonzero(xf == 0.0)
    n = xf.shape[0]
    starts = 1.0 if xf[0] != 0 else 0.0
    if zeros.size:
        nxt = zeros + 1
        nxt = nxt[nxt < n]
        starts += float(np.count_nonzero(xf[nxt] != 0.0))
    return starts


def kernel(inputs: np.ndarray, targets: np.ndarray) -> np.ndarray:
    import ml_dtypes
    from concourse.bass_utils import run_bass_kernel_spmd

    xf = np.ascontiguousarray(inputs, dtype=np.float32).reshape(-1)
    tf = np.ascontiguousarray(targets, dtype=np.float32).reshape(-1)
    assert xf.size == TOTAL and tf.size == TOTAL

    xb = xf.astype(ml_dtypes.bfloat16)
    tb = tf.astype(ml_dtypes.bfloat16)

    shard = TOTAL // N_CORES
    in_maps = []
    for c in range(N_CORES):
        in_maps.append({
            "x": xb[c * shard:(c + 1) * shard].reshape(P, COLS),
            "t": tb[c * shard:(c + 1) * shard].reshape(P, COLS),
        })

    nc = _get_nc()
    res = None
    for attempt in range(3):
        try:
            res = run_bass_kernel_spmd(nc, in_maps, core_ids=list(range(N_CORES)))
            break
        except Exception:
            if attempt == 2:
                raise
    assert res is not None

    s_xt = s_x = s_t = t1 = t2 = s_l2 = 0.0
    di = np.arange(SUBW)
    for c in range(N_CORES):
        psB = (res.results[c]["stats"].astype(np.float64)
               + res.results[c]["stats2"].astype(np.float64))
        t1 += psB[di, di].sum()                      # t.L1 diagonal
        t2 += psB[di, SUBW + di].sum()               # t.L2 diagonal
        s_xt += psB[di, 2 * SUBW + di].sum()         # t.x diagonal
        s_t += psB[0:SUBW, 3 * SUBW].sum()           # ones rhs column
        s_l2 += psB[SUBW, SUBW:2 * SUBW].sum()       # ones weight row
        s_x += psB[SUBW, 2 * SUBW:3 * SUBW].sum()
        # last 4 columns of this core's shard: done on host
        xl = xb[c * shard:(c + 1) * shard].reshape(P, COLS)[:, COLS_DEV:]
        tl = tb[c * shard:(c + 1) * shard].reshape(P, COLS)[:, COLS_DEV:]
        xl = xl.astype(np.float64).reshape(-1)
        tl = tl.astype(np.float64).reshape(-1)
        L1l = np.log(xl + B1)
        L2l = np.log((1.0 + EPS2) - xl)
        t1 += (tl * L1l).sum()
        t2 += (tl * L2l).sum()
        s_l2 += L2l.sum()
        s_x += xl.sum()
        s_t += tl.sum()
        s_xt += (xl * tl).sum()

    n_starts = _topology_starts(xf)

    dice = 1.0 - (2.0 * s_xt + SMOOTH) / (s_x + s_t + SMOOTH)
    bce = -(t1 - t2 + s_l2) / TOTAL
    topo = abs(n_starts - 1.0) / IMAGE_PIXELS
    loss = bce * BCE_WEIGHT + dice + topo * TOPOLOGY_WEIGHT
    return np.array(loss, dtype=np.float32)
